# revision 1
# baseline (speedup 1.0000x reference)
"""Trainium2 Bass kernel for CrossInnerProductWithBuyer.

Computes, per batch b (B=16384, E=128):
  out[b] = concat( windows[b] @ c[b],      # [10]
                   -(neg[b] @ c[b]),       # [64]
                   buy[b] @ c[b] )         # [1]
with c = center_vec.  Output [B, 75, 1] fp32.

Sharding: pure data-parallel over batch across 8 NeuronCores (2048
batches per core).  The host pre-transposes each core's shard so the
contraction axis e sits on the SBUF partition axis:

  at [E=128, BS*75]   columns ordered (b outer, r inner), r spanning
                      win(10) | neg(64) | buy(1)  == output order
  ct [E=128, BS]      center vectors, transposed

Per 128-batch tile the kernel then does:
  - DVE: three tensor_muls (win, neg, buy column groups) against a
    broadcast of ct / -ct  -> prod[e, (b, r)].  (The neg group uses -ct
    so the sign is folded into the product.)
  - PE:  ones[128,1]-stationary matmuls over N=512 column chunks:
    out[0, n] = sum_e prod[e, n] -- the e-reduction as a partition
    contraction.  Independent matmuls, no PSUM accumulation chains.
  - ACT: copies each PSUM strip [1, 512] to SBUF.
  - DMA: strips go out contiguously (column order == output row-major).

This keeps the DVE at exactly one pass over the data (its fp32
tensor_tensor floor), the reduction rides the otherwise-idle Tensor
engine, and GPSIMD stays idle (concurrent GPSIMD elementwise slows DVE
two-port ops ~3-5x, measured).
"""

import sys

if "/opt/trn_rl_repo" not in sys.path:
    sys.path.insert(0, "/opt/trn_rl_repo")

from contextlib import ExitStack

import numpy as np

import concourse.bass as bass
import concourse.mybir as mybir
import concourse.tile as tile
from concourse import bacc, bass_utils

B, W, N, E = 16384, 10, 64, 128
NCORES = 8
BS = B // NCORES            # 2048 batches per core
PT = 128                    # batches per tile
R = W + N + 1               # 75 output columns per batch
F = R * E                   # 9600 prod columns per tile
CHUNK = 512                 # matmul N (one PSUM bank of fp32)
STRIP = 2048                # PSUM strip: 4 chunks copied/stored together

FP32 = mybir.dt.float32


def _build(bs: int = BS) -> bass.Bass:
    nt = bs // PT
    nc = bacc.Bacc("TRN2", target_bir_lowering=False, debug=False,
                   num_devices=NCORES)
    at = nc.dram_tensor("at", [E, bs * R], FP32, kind="ExternalInput").ap()
    ct = nc.dram_tensor("ct", [E, bs], FP32, kind="ExternalInput").ap()
    out = nc.dram_tensor("out", [1, bs * R], FP32, kind="ExternalOutput").ap()

    with tile.TileContext(nc) as tc, ExitStack() as ctx:
        apool = ctx.enter_context(tc.tile_pool(name="a", bufs=4))
        cpool = ctx.enter_context(tc.tile_pool(name="c", bufs=4))
        ncpool = ctx.enter_context(tc.tile_pool(name="negc", bufs=4))
        spool = ctx.enter_context(tc.tile_pool(name="strip", bufs=3))
        pspool = ctx.enter_context(tc.tile_pool(name="ps", bufs=2,
                                                space="PSUM"))
        onepool = ctx.enter_context(tc.tile_pool(name="ones", bufs=1))

        ones = onepool.tile([E, 1], FP32)
        nc.vector.memset(ones[:], 1.0)

        for t in range(nt):
            col0 = t * F
            a = apool.tile([E, F], FP32)
            nc.sync.dma_start(a[:], at[:, col0:col0 + F])
            c = cpool.tile([E, PT], FP32)
            nc.sync.dma_start(c[:], ct[:, t * PT:(t + 1) * PT])
            negc = ncpool.tile([E, PT], FP32)
            nc.vector.tensor_scalar_mul(negc[:], c[:], -1.0)

            # a viewed as [e, b, r]; multiply r-groups by (+-)c[e, b],
            # in place (the product overwrites a, saving an SBUF buffer).
            av = a[:].rearrange("e (b r) -> e b r", r=R)
            p = a
            nc.vector.tensor_mul(
                av[:, :, 0:W], av[:, :, 0:W],
                c[:].unsqueeze(2).broadcast_to([E, PT, W]))
            nc.vector.tensor_mul(
                av[:, :, W:W + N], av[:, :, W:W + N],
                negc[:].unsqueeze(2).broadcast_to([E, PT, N]))
            nc.vector.tensor_mul(
                av[:, :, W + N:R], av[:, :, W + N:R],
                c[:].unsqueeze(2).broadcast_to([E, PT, 1]))

            # e-reduction on the Tensor engine: ones.T @ prod chunk.
            # 4 matmuls (N=512, one PSUM bank each) fill a 4-bank strip;
            # one ACT copy + one DMA per strip keeps the sem-chain short.
            for g0 in range(0, F, STRIP):
                gn = min(STRIP, F - g0)
                ps = pspool.tile([1, STRIP], FP32)
                for k0 in range(0, gn, CHUNK):
                    n = min(CHUNK, gn - k0)
                    nc.tensor.matmul(ps[:, k0:k0 + n], ones[:],
                                     p[:, g0 + k0:g0 + k0 + n],
                                     start=True, stop=True)
                s = spool.tile([1, STRIP], FP32)
                nc.scalar.copy(s[:, 0:gn], ps[:, 0:gn])
                nc.scalar.dma_start(out[:, col0 + g0:col0 + g0 + gn],
                                    s[:, 0:gn])
    nc.compile()
    return nc


_NC_CACHE: dict = {}


def _get_nc(bs: int = BS) -> bass.Bass:
    if bs not in _NC_CACHE:
        _NC_CACHE[bs] = _build(bs)
    return _NC_CACHE[bs]


def _prep_core(center, windows, negs, buy):
    """Transpose one core's shard to the kernel's (e-major) layout."""
    bs = center.shape[0]
    a = np.concatenate([
        windows.reshape(bs, W, E),
        negs.reshape(bs, N, E),
        buy.reshape(bs, 1, E),
    ], axis=1)                                   # [bs, 75, E]
    at = np.ascontiguousarray(a.transpose(2, 0, 1).reshape(E, bs * R),
                              dtype=np.float32)
    ct = np.ascontiguousarray(center.reshape(bs, E).T, dtype=np.float32)
    return at, ct


def _shard_inputs(center_vec, windows_vecs, neg_vecs, buy_vec):
    center_vec = np.asarray(center_vec, dtype=np.float32)
    windows_vecs = np.asarray(windows_vecs, dtype=np.float32)
    neg_vecs = np.asarray(neg_vecs, dtype=np.float32)
    buy_vec = np.asarray(buy_vec, dtype=np.float32)
    in_maps = []
    for i in range(NCORES):
        sl = slice(i * BS, (i + 1) * BS)
        at, ct = _prep_core(center_vec[sl], windows_vecs[sl],
                            neg_vecs[sl], buy_vec[sl])
        in_maps.append({"at": at, "ct": ct})
    return in_maps


def run(center_vec, windows_vecs, neg_vecs, buy_vec, trace: bool = False):
    """Run on 8 NeuronCores; returns (full_output, BassKernelResults)."""
    nc = _get_nc()
    in_maps = _shard_inputs(center_vec, windows_vecs, neg_vecs, buy_vec)
    res = bass_utils.run_bass_kernel_spmd(
        nc, in_maps, list(range(NCORES)), trace=trace)
    full = np.concatenate(
        [res.results[i]["out"].reshape(BS, R) for i in range(NCORES)], axis=0)
    return full.reshape(B, R, 1), res


def kernel(center_vec, windows_vecs, neg_vecs, buy_vec):
    out, _ = run(center_vec, windows_vecs, neg_vecs, buy_vec)
    return out

